# revision 13
# baseline (speedup 1.0000x reference)
"""Contrastive-learning loss on latent features — Trainium2 Bass kernel.

Math: x = act[:, :8].reshape(B, 256); mse[i,j] = ||x_i - x_j||^2 / D;
pos = relu(mse - tau_p) for same-label pairs, neg = relu(tau_n - mse) for
different-label pairs (diagonal excluded), each normalized by the pair
counts, summed, halved.

Device strategy (8 cores, batch rows sharded 1024/core after sorting rows
by label — the loss is permutation invariant):
Everything is folded into one PSUM accumulation per [128 x N] tile:
    v[i,j] = sq_i + sq_j - 2*x_i.x_j + W*[l_i == l_j]       (W = 1024)
via two matmuls: one fp8 DoubleRow matmul carrying the K=256 (-2x)^T x
Gram contribution in a single pass, and one bf16 K=12 chunk carrying
{-32*onehot(l)} x {-32*onehot(l)} = +1024*[l_i==l_j] plus rows encoding
sq_i*1 and 1*sq_j (sq hi/lo split across two bf16 rows for precision).
Then, in D-scaled units (thresholds scale by D):
    pos term = relu(v - A),  A = W + D*tau_p
    neg term = relu(Bc - v), Bc = D*tau_n
The W offset pushes the wrong branch of each relu below zero, so label
masking costs nothing; the matrix diagonal lands at v ~= W, which both
relus map to zero.  Each strip is row-sum-accumulated into per-slot
stats on ScalarE (Relu+bias(+scale -1)+accum) and VectorE (max/min+add+
accum), split between the engines to balance their throughput.  Because
rows are label-sorted, same-label pairs only occur within chunk distance
<= 10, so the pos pass runs only on the first 1408 columns of each row
subtile's 4224-column run.  The host applies slot weights and the final
normalization.

Symmetry: only ~half the pairwise matrix is computed.  With 64 global
row-chunks of 128, row-chunk R covers col-chunks (R+d) mod 64 for
d = 0..32; d=0 and d=32 blocks weigh 1, 1<=d<32 weigh 2.  Each core's
rhs columns are rotated by its row offset so all cores run the same
program over a 5120-wide column window.
"""

import numpy as np
import ml_dtypes

import concourse.bacc as bacc
import concourse.tile as tile
from concourse import mybir
from concourse.bass_utils import run_bass_kernel_spmd

B = 8192
D = 256
NCH = 8  # channels used from act
NLAB = 8
TAU_POS = 0.01
TAU_NEG = 1.0
W = 1024.0  # (-32)*(-32) label-equality offset
NCORES = 8
RPC = B // NCORES  # 1024 rows per core
NSUB = RPC // 128  # 8 row-subtiles per core (128 rows each)
DMAX = 32  # max chunk distance in the symmetric scheme
NCOLS = 128 * (NSUB - 1) + 128 * DMAX + 128  # 5120: rhs window per core
K2 = 12  # aux contraction chunk
A_POS = W + D * TAU_POS  # 1026.56
B_NEG = D * TAU_NEG  # 256.0
URUN = 4224  # run width per row-subtile (d = 0..32 -> 33 col-chunks)
BLKW = 512  # psum block width; 8 full blocks + 128-wide remainder per run
POS_COLS = 1408  # pos pass only for chunk distance d <= 10 (label-sorted)
# Max class size for which the d<=10 pos window is provably sufficient:
# j - i <= maxcount-1 <= 1216 -> chunk distance <= 10.
POS_SAFE_MAXCOUNT = 1217
USE_FP8 = True  # fp8e4m3 DoubleRow for the Gram matmul (else bf16, 2 chunks)

_BF16 = ml_dtypes.bfloat16
_FP8 = ml_dtypes.float8_e4m3


ACT_RATE = 0.735  # cols/ns used for static engine load balancing
DVE_RATE = 0.92


def _slot_table(pos_cols=POS_COLS):
    """Per-row-subtile slot layout: (block, piece_col0, ncols, weight,
    engine, kind) in emission order (must be grouped by block — the builder
    and postprocessor both walk this table in order).
    Blocks are BLKW wide (last one is the 128-col d=32 remainder).
    Weight: d=0 cols [0:128) of the run and the d=32 remainder weigh 1,
    everything else 2.  Pieces are assigned greedily to ScalarE/VectorE to
    balance their modeled throughput."""
    def pieces_for(kind, spans):
        out = []
        for (a0, a1, w) in spans:
            c = a0
            while c < a1:
                blk = c // BLKW
                end = min(a1, (blk + 1) * BLKW)
                out.append([kind, blk, c - blk * BLKW, end - c, w])
                c = end
        return out
    pos_spans = [(0, 128, 1.0), (128, min(pos_cols, URUN - 128), 2.0)]
    if pos_cols >= URUN:
        pos_spans.append((URUN - 128, URUN, 1.0))
    pos = pieces_for("pos", pos_spans)
    neg = pieces_for("neg", [(0, 128, 1.0), (128, URUN - 128, 2.0),
                             (URUN - 128, URUN, 1.0)])
    raw = sorted(pos + neg, key=lambda p: (p[1], p[2], p[0]))
    ta = td = 0.0
    out = []
    for kind, blk, c0, cn, w in raw:
        if ta + cn / ACT_RATE <= td + cn / DVE_RATE:
            out.append((blk, c0, cn, w, "act", kind))
            ta += cn / ACT_RATE
        else:
            out.append((blk, c0, cn, w, "dve", kind))
            td += cn / DVE_RATE
    out.sort(key=lambda s: s[0])
    return out


def _build_nc(slots_u=None):
    if slots_u is None:
        slots_u = _slot_table(POS_COLS)
    nslots = NSUB * len(slots_u)
    stats_w = 2 * nslots
    nc = bacc.Bacc("TRN2", target_bir_lowering=False, debug=False,
                   num_devices=NCORES)
    f32 = mybir.dt.float32
    bf16 = mybir.dt.bfloat16
    fp8 = mybir.dt.float8e4
    if USE_FP8:
        r0 = nc.dram_tensor("r0", [128, 2, NCOLS], fp8, kind="ExternalInput")
        l0 = nc.dram_tensor("l0", [128, 2, RPC], fp8, kind="ExternalInput")
    else:
        r0 = nc.dram_tensor("r0", [128, 2, NCOLS], bf16, kind="ExternalInput")
        l0 = nc.dram_tensor("l0", [128, 2, RPC], bf16, kind="ExternalInput")
    r2 = nc.dram_tensor("r2", [K2, NCOLS], bf16, kind="ExternalInput")
    l2 = nc.dram_tensor("l2", [K2, RPC], bf16, kind="ExternalInput")
    stats = nc.dram_tensor("stats", [128, stats_w], f32, kind="ExternalOutput")

    with tile.TileContext(nc) as tc:
        with (
            tc.tile_pool(name="big", bufs=1) as big,
            tc.tile_pool(name="consts", bufs=1) as consts,
            tc.tile_pool(name="psum", bufs=6, space="PSUM") as psum_pool,
            tc.tile_pool(name="scratch", bufs=4) as scratch,
        ):
            xdt = fp8 if USE_FP8 else bf16
            L0 = big.tile([128, 2, RPC], xdt)
            L2 = big.tile([K2, RPC], bf16)
            R2 = big.tile([K2, NCOLS], bf16)
            R0 = big.tile([128, 2, NCOLS], xdt)
            # order: lhs weights, then the first rhs chunk (so the first
            # Gram matmuls can start), then the small aux chunks, then the
            # rest of the rhs window
            nc.sync.dma_start(out=L0, in_=l0[:, :, :])
            bounds = [0, 640, 2133, 3626, NCOLS]
            sl = slice(bounds[0], bounds[1])
            nc.sync.dma_start(out=R0[:, :, sl], in_=r0[:, :, sl])
            nc.sync.dma_start(out=L2, in_=l2[:, :])
            nc.sync.dma_start(out=R2, in_=r2[:, :])
            for i in range(1, len(bounds) - 1):
                sl = slice(bounds[i], bounds[i + 1])
                nc.sync.dma_start(out=R0[:, :, sl], in_=r0[:, :, sl])

            bias_pos = consts.tile([128, 1], f32)
            nc.vector.memset(bias_pos, -A_POS)
            bias_neg = consts.tile([128, 1], f32)
            nc.vector.memset(bias_neg, B_NEG)
            act_stats = consts.tile([128, nslots], f32)
            dve_stats = consts.tile([128, nslots], f32)

            dr = mybir.MatmulPerfMode.DoubleRow if USE_FP8 else None
            slot = 0
            for u in range(NSUB):
                lsl = slice(128 * u, 128 * u + 128)
                base = 128 * u
                for blk in range((URUN + BLKW - 1) // BLKW):
                    wid = min(BLKW, URUN - BLKW * blk)
                    bc0 = base + BLKW * blk
                    ps = psum_pool.tile([128, wid], f32, tag="ps")
                    for s0 in range(0, wid, 512):
                        sw = min(512, wid - s0)
                        csl = slice(bc0 + s0, bc0 + s0 + sw)
                        if USE_FP8:
                            nc.tensor.matmul(
                                ps[:, s0:s0 + sw], L0[:, :, lsl],
                                R0[:, :, csl], start=True, stop=False,
                                perf_mode=dr)
                        else:
                            nc.tensor.matmul(
                                ps[:, s0:s0 + sw], L0[:, 0, lsl],
                                R0[:, 0, csl], start=True, stop=False)
                            nc.tensor.matmul(
                                ps[:, s0:s0 + sw], L0[:, 1, lsl],
                                R0[:, 1, csl], start=False, stop=False)
                        nc.tensor.matmul(ps[:, s0:s0 + sw], L2[:, lsl],
                                         R2[:, csl], start=False, stop=True)
                    for (s_blk, p_c0, cn, w, eng, kind) in slots_u:
                        if s_blk != blk:
                            continue
                        src = ps[:, p_c0:p_c0 + cn]
                        if eng == "act":
                            acc = act_stats[:, slot:slot + 1]
                            o = scratch.tile([128, BLKW], f32, tag="actout")
                            if kind == "pos":
                                nc.scalar.activation(
                                    out=o[:, :cn], in_=src,
                                    func=mybir.ActivationFunctionType.Relu,
                                    bias=bias_pos, scale=1.0, accum_out=acc)
                            else:
                                nc.scalar.activation(
                                    out=o[:, :cn], in_=src,
                                    func=mybir.ActivationFunctionType.Relu,
                                    bias=bias_neg, scale=-1.0, accum_out=acc)
                        else:
                            acc = dve_stats[:, slot:slot + 1]
                            o = scratch.tile([128, BLKW], f32, tag="dveout")
                            # NB: in accumulate mode scalar2 is added ONCE
                            # per partition to the final sum, not per element
                            if kind == "pos":
                                nc.vector.tensor_scalar(
                                    out=o[:, :cn], in0=src,
                                    scalar1=A_POS, scalar2=-float(cn) * A_POS,
                                    op0=mybir.AluOpType.max,
                                    op1=mybir.AluOpType.add, accum_out=acc)
                            else:  # accum = -sum(relu(B_NEG - v))
                                nc.vector.tensor_scalar(
                                    out=o[:, :cn], in0=src,
                                    scalar1=B_NEG, scalar2=-float(cn) * B_NEG,
                                    op0=mybir.AluOpType.min,
                                    op1=mybir.AluOpType.add, accum_out=acc)
                        slot += 1
            assert slot == nslots, slot
            nc.sync.dma_start(out=stats[:, :nslots], in_=act_stats)
            nc.sync.dma_start(out=stats[:, nslots:], in_=dve_stats)
    nc.compile()
    return nc


def _prep_inputs(act: np.ndarray, labels: np.ndarray, order: np.ndarray):
    x = np.ascontiguousarray(act[:, :NCH, :]).reshape(B, D).astype(np.float32)
    x = x[order]
    lab = labels[order]
    xdt = _FP8 if USE_FP8 else _BF16
    xb = x.astype(xdt)
    xb32 = xb.astype(np.float32)
    # sq from the ORIGINAL x: keeps the pairwise mse unbiased under the fp8
    # Gram rounding (the cross term is mean-zero noise).  The diagonal then
    # deviates from 0 by ~|sq - sq(xhat)|, which stays far below the W
    # offset and only negligibly leaks past the D*tau_p relu threshold.
    sq = (x * x).sum(axis=1)  # [B] f32
    sq_hi = sq.astype(_BF16)
    sq_lo = (sq - sq_hi.astype(np.float32)).astype(_BF16)
    oh = (lab.reshape(-1, 1) == np.arange(NLAB).reshape(1, -1))
    ohm = (-32.0 * oh.astype(np.float32)).astype(_BF16)  # [B, 8]

    ones = np.ones(B, dtype=_BF16)
    # Gram operands as [128, 2, B]: contraction dim d = 2*k + j
    R0g = np.ascontiguousarray(xb.T.reshape(128, 2, B))
    L0g = np.ascontiguousarray((-2.0 * xb32.T).astype(xdt).reshape(128, 2, B))
    R2g = np.empty((K2, B), dtype=_BF16)
    R2g[:NLAB] = ohm.T
    R2g[8] = ones
    R2g[9] = ones
    R2g[10] = sq_hi
    R2g[11] = sq_lo
    L2g = np.empty((K2, B), dtype=_BF16)
    L2g[:NLAB] = ohm.T
    L2g[8] = sq_hi
    L2g[9] = sq_lo
    L2g[10] = ones
    L2g[11] = ones

    in_maps = []
    for c in range(NCORES):
        cols = (RPC * c + np.arange(NCOLS)) % B
        rows = slice(RPC * c, RPC * (c + 1))
        in_maps.append({
            "r0": np.ascontiguousarray(R0g[:, :, cols]),
            "r2": np.ascontiguousarray(R2g[:, cols]),
            "l0": np.ascontiguousarray(L0g[:, :, rows]),
            "l2": np.ascontiguousarray(L2g[:, rows]),
        })
    return in_maps


def _postprocess(results, labels: np.ndarray, slots_u) -> np.float32:
    nslots = NSUB * len(slots_u)
    s_pos = 0.0
    s_neg = 0.0
    for c in range(NCORES):
        st = results[c]["stats"].astype(np.float64)
        slot = 0
        for u in range(NSUB):
            for (_, _, cn, w, eng, kind) in slots_u:
                col = slot if eng == "act" else nslots + slot
                v = st[:, col].sum()
                if kind == "pos":
                    s_pos += w * v
                elif eng == "act":  # act neg accumulates +sum(relu(B-v))
                    s_neg += w * v
                else:  # dve neg accumulates -sum(relu(B-v))
                    s_neg += w * (-v)
                slot += 1
    s_pos /= D
    s_neg /= D
    cnt = np.bincount(labels.astype(np.int64), minlength=NLAB).astype(np.float64)
    c_pos = (cnt * (cnt - 1.0)).sum() / 2.0
    n_pairs = B * (B - 1) / 2.0
    c_neg = n_pairs - c_pos
    loss = (s_pos / c_pos + s_neg / c_neg) / 2.0
    return np.float32(loss)


_NC_CACHE = {}


def kernel(act: np.ndarray, labels: np.ndarray) -> np.ndarray:
    lab = labels.astype(np.int64).reshape(-1)
    # The narrow pos window relies on label-sorted rows having class spans
    # <= 10 chunks; fall back to full pos coverage for pathological labels.
    if np.bincount(lab, minlength=NLAB).max() <= POS_SAFE_MAXCOUNT:
        key = "narrow"
        slots_u = _slot_table(POS_COLS)
    else:
        key = "full"
        slots_u = _slot_table(URUN)
    order = np.argsort(lab, kind="stable")
    if key not in _NC_CACHE:
        _NC_CACHE[key] = _build_nc(slots_u)
        _NC_CACHE.setdefault("nc", _NC_CACHE[key])  # for test harness use
    nc = _NC_CACHE[key]
    in_maps = _prep_inputs(act, lab, order)
    res = run_bass_kernel_spmd(nc, in_maps, core_ids=list(range(NCORES)))
    return np.array(_postprocess(res.results, lab, slots_u), dtype=np.float32)
